# revision 1
# baseline (speedup 1.0000x reference)
"""Causal single-head attention on 8 TRN2 NeuronCores.

Problem: x [4, 2048, 768] f32; Wq/Wk/Wv [768, 768] f32 (torch Linear layout).
  q/k/v = x @ W.T ; scores = q k^T causal-masked; attn = softmax(scores/sqrt(768));
  out = attn @ v.

Sharding: core c -> batch b = c//2, half h = c%2. The two cores of a batch
split the 16 query tiles (128 rows each) INTERLEAVED: core h owns global
q-tiles {2*lt + h : lt in 0..7}. Causal attention for global q-tile g only
needs keys 0 .. 128*(g+1), i.e. ceil((g+1)/4) 512-wide key chunks; with the
even/odd interleave both cores see the identical chunk-count sequence
[1,1,2,2,3,3,4,4], so the SPMD program is uniform across cores while doing
EXACT causal work (no fully-masked chunks are ever computed). Only the
diagonal chunk of each q-tile needs masking; its 4 possible within-chunk
patterns are passed as a small per-core strip input ([128, 1024], window
picked by lt%2).

The host passes x^T (global key order, shared by the pair), xq^T (the core's
own interleaved query rows), and W^T - host transposes are pure layout prep.
Matmuls run in float32r (TensorE fast-fp32, 4x the fp32 rate at free dim
>= 256, ~2e-4 input rounding); raw fp32 bits feed float32r-typed DRAM inputs
directly - the PE converts on load, so the device does no transposes and no
rounding work at all.

Device pipeline per core:
  1. Q^T [768,1024] projected first (resident in SBUF), then stream x^T in
     512-col chunks -> K^T [768,2048] and V [2048,768] resident in SBUF;
     attention tiles can begin as soon as the first K/V chunks land.
  2. Per local q-tile lt (Nc = [1,1,2,2,3,3,4,4][lt] key chunks): scores via
     f32r matmuls; non-diagonal chunks exp directly from PSUM on ScalarE;
     the diagonal chunk gets a VectorE strip-add (fused evacuation) then exp;
     every exp emits its row-sum via accum_out (no max-subtraction: scaled
     scores are O(+-5), safely inside fp32 exp range). attn tiles transposed
     on TensorE 4-per-PSUM-bank; context accumulates over 4*Nc key tiles;
     softmax 1/rowsum is fused into the context PSUM evacuation.
"""

import os
import sys
from contextlib import ExitStack

import numpy as np

for _p in ("/opt/trn_rl_repo", "/root/.axon_site/_ro/trn_rl_repo"):
    if os.path.isdir(_p) and _p not in sys.path:
        sys.path.append(_p)

import concourse.mybir as mybir  # noqa: E402
import concourse.tile as tile  # noqa: E402
from concourse import bacc  # noqa: E402
from concourse.bass_utils import run_bass_kernel_spmd  # noqa: E402
from concourse.masks import make_identity  # noqa: E402

F32 = mybir.dt.float32
F32R = mybir.dt.float32r

BATCH = 4
SEQ = 2048
D = 768
DK = D // 128  # contraction chunks (6)
NQ = 1024  # query rows per core
LT = NQ // 128  # local q-tiles per core (8)
XC = 512  # streaming chunk width
NCS = [1, 1, 2, 2, 3, 3, 4, 4]  # key chunks per local q-tile (both cores!)
SCALE = 1.0 / float(np.sqrt(np.float32(D)))
NEG = -1e30

_CACHE = {}


def _build(repeat=1):
    nc = bacc.Bacc("TRN2", target_bir_lowering=False, debug=False, num_devices=8)
    xt_d = nc.declare_dram_parameter("xt", [D, SEQ], F32R, isOutput=False)
    xqt_d = nc.declare_dram_parameter("xqt", [D, NQ], F32R, isOutput=False)
    wqt_d = nc.declare_dram_parameter("wqt", [D, D], F32R, isOutput=False)
    wkt_d = nc.declare_dram_parameter("wkt", [D, D], F32R, isOutput=False)
    wvt_d = nc.declare_dram_parameter("wvt", [D, D], F32R, isOutput=False)
    strip_d = nc.declare_dram_parameter("strip", [128, 1024], F32, isOutput=False)
    out_d = nc.declare_dram_parameter("out", [NQ, D], F32, isOutput=True)

    # Rotate input DMAs across engines' DGE queues - a single queue serializes
    # the ~16MB of input transfers and stalls the PE at kernel start.
    _dma_i = [0]

    def dma_in(dst, src):
        eng = (nc.sync, nc.scalar)[_dma_i[0] % 2]
        eng.dma_start(dst, src)
        _dma_i[0] += 1

    # Round-robin PSUM evacuation between VectorE and ScalarE.
    _evac_i = [0]

    def evac(dst, src):
        if _evac_i[0] % 2 == 0:
            nc.vector.tensor_copy(dst, src)
        else:
            nc.scalar.copy(dst, src)
        _evac_i[0] += 1

    with tile.TileContext(nc) as tc, ExitStack() as ctx:
        persist = ctx.enter_context(tc.tile_pool(name="persist", bufs=1))

        ident = persist.tile([128, 128], F32)
        make_identity(nc, ident[:])

        strip = persist.tile([128, 1024], F32)
        nc.gpsimd.dma_start(strip[:], strip_d[:])

        kt = persist.tile([128, DK, SEQ], F32R)  # K^T
        vt = persist.tile([128, SEQ // 128, D], F32R)  # V (natural layout)
        qt_sb = persist.tile([128, DK, NQ], F32R)  # Q^T (resident)

        for _rep in range(repeat):
          # wk prefetches in a pool coexisting with the whole Q phase, so its
          # DMA is not blocked on the Q-phase SBUF region being released.
          with ExitStack() as p1:
            wkpool = p1.enter_context(tc.tile_pool(name="wkpool", bufs=1))
            wtk = wkpool.tile([128, DK, D], F32R, name="wtk")

            # ---------------- Phase 1b: Q^T projection (resident) ----------------
            with ExitStack() as p2b:
                wqpool = p2b.enter_context(tc.tile_pool(name="wqpool", bufs=1))
                xqc_p = p2b.enter_context(tc.tile_pool(name="xqc", bufs=2))
                ps_q = p2b.enter_context(
                    tc.tile_pool(name="ps_q", bufs=3, space="PSUM")
                )
                # wtq/xqc split into ko-halves as SEPARATE tiles: dependency
                # tracking is per-tile, so ko 0..2 matmuls start after half the
                # input bytes instead of waiting for the full load.
                wtqh = []
                for half in range(2):
                    wq_h = wqpool.tile([128, 3, D], F32R, name=f"wtq{half}")
                    dma_in(
                        wq_h[:],
                        wqt_d[half * 384 : (half + 1) * 384, :].rearrange(
                            "(ko p) o -> p ko o", p=128
                        ),
                    )
                    wtqh.append(wq_h)
                for sc in range(NQ // XC):
                    xqch = []
                    for half in range(2):
                        xq_h = xqc_p.tile([128, 3, XC], F32R, tag=f"xqc{half}")
                        dma_in(
                            xq_h[:],
                            xqt_d[
                                half * 384 : (half + 1) * 384,
                                sc * XC : (sc + 1) * XC,
                            ].rearrange("(ko p) s -> p ko s", p=128),
                        )
                        xqch.append(xq_h)
                    if sc == 0:
                        # prefetch W_k during the Q phase
                        for half in range(2):
                            dma_in(
                                wtk[:, half * 3 : (half + 1) * 3, :],
                                wkt_d[half * 384 : (half + 1) * 384, :].rearrange(
                                    "(ko p) o -> p ko o", p=128
                                ),
                            )
                    for oo in range(DK):
                        pq = ps_q.tile([128, XC], F32, tag="ps_q")
                        for ko in range(DK):
                            nc.tensor.matmul(
                                pq[:],
                                wtqh[ko // 3][:, ko % 3, oo * 128 : (oo + 1) * 128],
                                xqch[ko // 3][:, ko % 3, :],
                                start=(ko == 0),
                                stop=(ko == DK - 1),
                            )
                        nc.vector.tensor_copy(qt_sb[:, oo, sc * XC : (sc + 1) * XC], pq[:])

            # ---------------- Phase 1a: K^T / V projections ----------------
            with ExitStack() as p2:
                wvpool = p2.enter_context(tc.tile_pool(name="wvpool", bufs=1))
                xtc_p = p2.enter_context(tc.tile_pool(name="xtc", bufs=2))
                ps_p512 = p2.enter_context(
                    tc.tile_pool(name="ps_p512", bufs=3, space="PSUM")
                )
                ps_p384 = p2.enter_context(
                    tc.tile_pool(name="ps_p384", bufs=4, space="PSUM")
                )

                wtv = wvpool.tile([128, DK, D], F32R, name="wtv")

                for sc in range(SEQ // XC):
                    xtc = xtc_p.tile([128, DK, XC], F32R, tag="xtc")
                    for half in range(2):
                        dma_in(
                            xtc[:, half * 3 : (half + 1) * 3, :],
                            xt_d[
                                half * 384 : (half + 1) * 384,
                                sc * XC : (sc + 1) * XC,
                            ].rearrange("(ko p) s -> p ko s", p=128),
                        )
                    if sc == 0:
                        # W_v load queues behind xtc0 so K-chunk0 starts sooner
                        for half in range(2):
                            dma_in(
                                wtv[:, half * 3 : (half + 1) * 3, :],
                                wvt_d[half * 384 : (half + 1) * 384, :].rearrange(
                                    "(ko p) o -> p ko o", p=128
                                ),
                            )

                    # K^T chunk
                    for oo in range(DK):
                        pk = ps_p512.tile([128, XC], F32, tag="p512")
                        for ko in range(DK):
                            nc.tensor.matmul(
                                pk[:],
                                wtk[:, ko, oo * 128 : (oo + 1) * 128],
                                xtc[:, ko, :],
                                start=(ko == 0),
                                stop=(ko == DK - 1),
                            )
                        evac(kt[:, oo, sc * XC : (sc + 1) * XC], pk[:])

                    # V chunk: per 128-row seq tile, dout in two 384 halves
                    for st in range(XC // 128):
                        seq_tile = sc * (XC // 128) + st
                        for oc in range(2):
                            pv = ps_p384.tile([128, 384], F32, tag="p384")
                            for ko in range(DK):
                                nc.tensor.matmul(
                                    pv[:],
                                    xtc[:, ko, st * 128 : (st + 1) * 128],
                                    wtv[:, ko, oc * 384 : (oc + 1) * 384],
                                    start=(ko == 0),
                                    stop=(ko == DK - 1),
                                )
                            evac(vt[:, seq_tile, oc * 384 : (oc + 1) * 384], pv[:])

            # ---------------- Phase 2: attention per local q-tile ----------------
            with ExitStack() as p3:
                scd_p = p3.enter_context(tc.tile_pool(name="scd", bufs=3))
                attn_p = p3.enter_context(tc.tile_pool(name="attn", bufs=3))
                attnT_p = p3.enter_context(tc.tile_pool(name="attnT", bufs=3))
                ctx_p = p3.enter_context(tc.tile_pool(name="ctxs", bufs=3))
                small_p = p3.enter_context(tc.tile_pool(name="small", bufs=2))
                ps_s = p3.enter_context(tc.tile_pool(name="ps_s", bufs=3, space="PSUM"))
                ps_t3 = p3.enter_context(
                    tc.tile_pool(name="ps_t3", bufs=2, space="PSUM")
                )
                ps_c1 = p3.enter_context(
                    tc.tile_pool(name="ps_c1", bufs=2, space="PSUM")
                )
                ps_c2 = p3.enter_context(
                    tc.tile_pool(name="ps_c2", bufs=1, space="PSUM")
                )

                for lt in range(LT):
                    ncs = NCS[lt]
                    attn = attn_p.tile([128, SEQ], F32, tag="attn")
                    rs = small_p.tile([128, 4], F32, tag="rs")

                    for kc in range(ncs):
                        pss = ps_s.tile([128, 512], F32, tag="ps_s")
                        for ko in range(DK):
                            nc.tensor.matmul(
                                pss[:],
                                qt_sb[:, ko, lt * 128 : (lt + 1) * 128],
                                kt[:, ko, kc * 512 : (kc + 1) * 512],
                                start=(ko == 0),
                                stop=(ko == DK - 1),
                            )
                        if kc == ncs - 1:
                            # diagonal chunk: strip-add (VectorE, fused evac), then exp
                            scd = scd_p.tile([128, 512], F32, tag="scd")
                            nc.vector.tensor_add(
                                scd[:],
                                pss[:],
                                strip[:, (lt % 2) * 512 : (lt % 2) * 512 + 512],
                            )
                            nc.scalar.activation(
                                attn[:, kc * 512 : (kc + 1) * 512],
                                scd[:],
                                mybir.ActivationFunctionType.Exp,
                                scale=SCALE,
                                accum_out=rs[:, kc : kc + 1],
                            )
                        else:
                            # interior chunk: exp straight from PSUM
                            nc.scalar.activation(
                                attn[:, kc * 512 : (kc + 1) * 512],
                                pss[:],
                                mybir.ActivationFunctionType.Exp,
                                scale=SCALE,
                                accum_out=rs[:, kc : kc + 1],
                            )

                    attnT = attnT_p.tile([128, SEQ // 128, 128], F32R, tag="attnT")
                    for kc in range(ncs):
                        pst = ps_t3.tile([128, 512], F32, tag="ps_t3")
                        for t in range(4):
                            nc.tensor.matmul(
                                pst[:, t * 128 : (t + 1) * 128],
                                attn[:, (kc * 4 + t) * 128 : (kc * 4 + t + 1) * 128],
                                ident[:],
                                is_transpose=True,
                                start=(t == 0),
                                stop=(t == 3),
                            )
                        nc.vector.tensor_copy(attnT[:, kc * 4 : kc * 4 + 4, :], pst[:])

                    nkt = 4 * ncs
                    pc1 = ps_c1.tile([128, 512], F32, tag="ps_c1")
                    pc2 = ps_c2.tile([128, 256], F32, tag="ps_c2")
                    for ktile in range(nkt):
                        nc.tensor.matmul(
                            pc1[:],
                            attnT[:, ktile, :],
                            vt[:, ktile, 0:512],
                            start=(ktile == 0),
                            stop=(ktile == nkt - 1),
                        )
                    for ktile in range(nkt):
                        nc.tensor.matmul(
                            pc2[:],
                            attnT[:, ktile, :],
                            vt[:, ktile, 512:768],
                            start=(ktile == 0),
                            stop=(ktile == nkt - 1),
                        )

                    rsum = small_p.tile([128, 1], F32, tag="rsum")
                    nc.vector.reduce_sum(
                        rsum[:], rs[:, 0:ncs], axis=mybir.AxisListType.X
                    )
                    rinv = small_p.tile([128, 1], F32, tag="rinv")
                    nc.vector.reciprocal(rinv[:], rsum[:])

                    ctx_sb = ctx_p.tile([128, D], F32, tag="ctxs")
                    nc.vector.tensor_mul(
                        ctx_sb[:, 0:512], pc1[:], rinv[:].to_broadcast((128, 512))
                    )
                    nc.vector.tensor_mul(
                        ctx_sb[:, 512:768], pc2[:], rinv[:].to_broadcast((128, 256))
                    )
                    nc.sync.dma_start(out_d[lt * 128 : (lt + 1) * 128, :], ctx_sb[:])

    nc.compile()
    return nc


def _strip_variant(v):
    """Within-chunk causal mask for a diagonal chunk of residue v = g mod 4:
    allow key jj (0..511) for row i iff jj <= 128*v + i."""
    i = np.arange(128)[:, None]
    jj = np.arange(512)[None, :]
    return np.where(jj <= 128 * v + i, 0.0, NEG).astype(np.float32)


def kernel(x, Wq, Wk, Wv):
    if "nc" not in _CACHE:
        _CACHE["nc"] = _build()
    nc = _CACHE["nc"]

    x = np.ascontiguousarray(x, dtype=np.float32)
    wqt = np.ascontiguousarray(np.asarray(Wq, dtype=np.float32).T)
    wkt = np.ascontiguousarray(np.asarray(Wk, dtype=np.float32).T)
    wvt = np.ascontiguousarray(np.asarray(Wv, dtype=np.float32).T)

    in_maps = []
    for c in range(8):
        b, h = c // 2, c % 2
        xb = x[b]
        # own query rows: global q-tiles 2*lt + h
        own = np.concatenate(
            [xb[(2 * lt + h) * 128 : (2 * lt + h + 1) * 128] for lt in range(LT)],
            axis=0,
        )
        # strip windows: lt%2==0 -> variant h; lt%2==1 -> variant 2+h
        strip = np.concatenate([_strip_variant(h), _strip_variant(2 + h)], axis=1)
        in_maps.append(
            {
                "xt": np.ascontiguousarray(xb.T),
                "xqt": np.ascontiguousarray(own.T),
                "wqt": wqt,
                "wkt": wkt,
                "wvt": wvt,
                "strip": np.ascontiguousarray(strip),
            }
        )

    res = run_bass_kernel_spmd(
        nc,
        in_maps,
        list(range(8)),
        trace=bool(int(os.environ.get("KERNEL_TRACE", "0"))),
    )
    _CACHE["last_results"] = res

    out = np.empty((BATCH, SEQ, D), np.float32)
    for c in range(8):
        b, h = c // 2, c % 2
        o = res.results[c]["out"]
        for lt in range(LT):
            out[b, (2 * lt + h) * 128 : (2 * lt + h + 1) * 128] = o[
                lt * 128 : (lt + 1) * 128
            ]
    return out



# revision 6
# speedup vs baseline: 1.6530x; 1.6530x over previous
"""Causal single-head attention on 8 TRN2 NeuronCores — fp8 DoubleRow version.

Problem: x [4, 2048, 768] f32; Wq/Wk/Wv [768, 768] f32 (torch Linear layout).
  q/k/v = x @ W.T ; scores = q k^T causal-masked; attn = softmax(scores/sqrt(768));
  out = attn @ v.

Sharding: core c -> batch b = c//2, half h = c%2. The two cores of a batch
split the 16 query tiles (128 rows each) interleaved: core h owns global
q-tiles {2*lt + h}. The host permutes x^T's columns per-core so that within
each 512-column chunk the core's OWN two q-tiles come first:
  chunk sc columns = global tiles [4sc+h, 4sc+2+h, 4sc+1-h, 4sc+3-h].
This makes the Q projection a fixed [0:256] slice of each chunk (SPMD-uniform
across cores) while K/V simply inherit the permuted key order, which both
attention phases use consistently. Causal masking becomes per-core strip DATA:
by construction key-tile position parity determines diagonal / fully-masked /
fully-allowed, identical program on every core.

Numerics: all matmuls run in fp8-e4m3 with the DoubleRow perf mode (two
128-deep contraction tiles per instruction at 2x rate). Projections use a
same-scale hi+lo fp8 split of both x and W (x ~ (xh+xl)/16, W ~ (wh+wl)/512)
and accumulate the three significant cross terms in one PSUM group, giving
~1e-3 relative error. Scores quantize q,k to fp8 at scale 32 (error ~7e-3).
Softmax skips max-subtraction (scaled scores are O(+-2)) and folds all scale
constants into the exp scale. The context matmul runs in f32r from the
transposed attention weights written directly by exp (scores are computed
pre-transposed: S^T = K Q^T, keys on partitions), so no PE transposes exist.
The softmax row-sum comes from a ones-column appended to V, accumulated in
the same PSUM as the context, and is divided out at evacuation.

Attention is exact-causal at 128-key granularity: key-tile t is scored only
against the query range that can attend to it (plus one fully-masked 128-wide
block on even cores to keep the instruction stream uniform).
"""

import os
import sys
from contextlib import ExitStack

import numpy as np
from ml_dtypes import float8_e4m3

for _p in ("/opt/trn_rl_repo", "/root/.axon_site/_ro/trn_rl_repo"):
    if os.path.isdir(_p) and _p not in sys.path:
        sys.path.append(_p)

import concourse.mybir as mybir  # noqa: E402
import concourse.tile as tile  # noqa: E402
from concourse import bacc  # noqa: E402
from concourse.bass_utils import run_bass_kernel_spmd  # noqa: E402

F32 = mybir.dt.float32
F32R = mybir.dt.float32r
F8 = mybir.dt.float8e4
DR = mybir.MatmulPerfMode.DoubleRow
EXP = mybir.ActivationFunctionType.Exp

BATCH = 4
SEQ = 2048
D = 768
NQ = 1024  # query rows per core
NEG = -1e30

SX = 16.0  # x fp8 scale
SW = 512.0  # W fp8 scale
SQK = 32.0  # q/k fp8 scale
S_PROJ = SX * SW  # PSUM scale of projections
EV_QK = SQK / S_PROJ  # evac scale PSUM -> q/k fp8
EV_V = 1.0 / S_PROJ  # evac scale PSUM -> v f32
SC_EXP = 1.0 / (float(np.sqrt(np.float32(D))) * SQK * SQK)

# key-tile position p within a chunk -> min local q-tile offset (2sc + MOFF[p])
MOFF = (0, 1, 0, 1)


def _mt(t):  # min local q-tile index attending to key-tile t
    return 2 * (t // 4) + MOFF[t % 4]


_W = [NQ - 128 * _mt(t) for t in range(16)]  # scored q-width per key-tile
_OFF = [0] * 16  # attnT column offset per key-tile
for _t in range(1, 16):
    _OFF[_t] = _OFF[_t - 1] + _W[_t - 1]
ATTNT_COLS = _OFF[15] + _W[15]  # 9216

_CACHE = {}


def _pieces(qs):
    """Split q-range [qs, NQ) into the strip piece (128) + <=512 chunks."""
    out = [(qs, 128)]
    pos = qs + 128
    while pos < NQ:
        w = min(512, NQ - pos)
        out.append((pos, w))
        pos += w
    return out


def _build():
    nc = bacc.Bacc("TRN2", target_bir_lowering=False, debug=False, num_devices=8)
    xh_d = nc.declare_dram_parameter("xh", [D, SEQ], F8, isOutput=False)
    xl_d = nc.declare_dram_parameter("xl", [D, SEQ], F8, isOutput=False)
    w_d = {}
    for w in ("wq", "wk", "wv"):
        for p in ("h", "l"):
            w_d[w + p] = nc.declare_dram_parameter(w + p, [D, D], F8, isOutput=False)
    strip_d = nc.declare_dram_parameter("strip", [128, 256], F32, isOutput=False)
    out_d = nc.declare_dram_parameter("out", [NQ, D], F32, isOutput=True)

    # Rotate input DMAs across engine DGE queues (issue-side seq cost).
    _dma_i = [0]

    def dma_in(dst, src):
        eng = (nc.sync, nc.scalar)[_dma_i[0] % 2]
        eng.dma_start(dst, src)
        _dma_i[0] += 1

    with tile.TileContext(nc) as tc, ExitStack() as ctx:
        persist = ctx.enter_context(tc.tile_pool(name="persist", bufs=1))

        strip = persist.tile([128, 256], F32)
        kt8 = persist.tile([128, 6, SEQ], F8)  # K^T fp8 (scale SQK)
        qt8 = persist.tile([128, 6, NQ], F8)  # Q^T fp8 (scale SQK)
        vt = persist.tile([128, 16, 776], F32R)  # V (+ones col 768)
        attnT = persist.tile([128, ATTNT_COLS], F32R)  # exp(S^T) blocks

        wt = {}
        for w in ("wq", "wk", "wv"):
            for p in ("h", "l"):
                wt[w + p] = persist.tile([128, 6, D], F8, name=w + p)

        ones = persist.tile([128, 1], F32)
        nc.vector.memset(ones[:], 1.0)
        nc.vector.tensor_copy(vt[:, :, 768:770], ones[:].to_broadcast((128, 16, 2)))

        def dma_w(name):
            dma_in(
                wt[name][:],
                w_d[name][:].rearrange("(ko p) o -> p ko o", p=128),
            )

        # ---------------- Phase 1: projections ----------------
        with ExitStack() as p1:
            xc_p = p1.enter_context(tc.tile_pool(name="xc", bufs=2))
            ps_q = p1.enter_context(tc.tile_pool(name="ps_q", bufs=2, space="PSUM"))
            ps_k = p1.enter_context(tc.tile_pool(name="ps_k", bufs=2, space="PSUM"))
            ps_v1 = p1.enter_context(tc.tile_pool(name="ps_v1", bufs=2, space="PSUM"))
            ps_v2 = p1.enter_context(tc.tile_pool(name="ps_v2", bufs=2, space="PSUM"))

            dma_w("wqh")

            for sc in range(4):
                xh = xc_p.tile([128, 6, 512], F8, tag="xh")
                xl = xc_p.tile([128, 6, 512], F8, tag="xl")
                # pair-granular loads so the first Q group starts early
                for j in range(3):
                    dma_in(
                        xh[:, 2 * j : 2 * j + 2, :],
                        xh_d[
                            256 * j : 256 * (j + 1), 512 * sc : 512 * (sc + 1)
                        ].rearrange("(ko p) s -> p ko s", p=128),
                    )
                if sc == 0:
                    dma_w("wql")
                for j in range(3):
                    dma_in(
                        xl[:, 2 * j : 2 * j + 2, :],
                        xl_d[
                            256 * j : 256 * (j + 1), 512 * sc : 512 * (sc + 1)
                        ].rearrange("(ko p) s -> p ko s", p=128),
                    )
                if sc == 0:
                    dma_w("wkh")
                    dma_w("wkl")
                    dma_w("wvh")
                    dma_w("wvl")
                    dma_in(strip[:], strip_d[:])

                terms_qk = ((xh, "h"), (xl, "h"), (xh, "l"))

                # Q: own q-tiles live in chunk cols [0:256]
                for oo in range(6):
                    pq = ps_q.tile([128, 256], F32, tag="pq")
                    for ti, (xa, wp) in enumerate(terms_qk):
                        for j in range(3):
                            nc.tensor.matmul(
                                pq[:],
                                wt["wq" + wp][:, 2 * j : 2 * j + 2, 128 * oo : 128 * (oo + 1)],
                                xa[:, 2 * j : 2 * j + 2, 0:256],
                                start=(ti == 0 and j == 0),
                                stop=(ti == 2 and j == 2),
                                perf_mode=DR,
                            )
                    nc.vector.tensor_scalar_mul(
                        qt8[:, oo, 256 * sc : 256 * (sc + 1)], pq[:], EV_QK
                    )

                # K^T
                for oo in range(6):
                    pk = ps_k.tile([128, 512], F32, tag="pk")
                    for ti, (xa, wp) in enumerate(terms_qk):
                        for j in range(3):
                            nc.tensor.matmul(
                                pk[:],
                                wt["wk" + wp][:, 2 * j : 2 * j + 2, 128 * oo : 128 * (oo + 1)],
                                xa[:, 2 * j : 2 * j + 2, :],
                                start=(ti == 0 and j == 0),
                                stop=(ti == 2 and j == 2),
                                perf_mode=DR,
                            )
                    nc.scalar.activation(
                        kt8[:, oo, 512 * sc : 512 * (sc + 1)],
                        pk[:],
                        mybir.ActivationFunctionType.Copy,
                        scale=EV_QK,
                    )

                # V rows (natural layout), d_out in 512+256
                for st in range(4):
                    seq_tile = 4 * sc + st
                    pv1 = ps_v1.tile([128, 512], F32, tag="pv1")
                    pv2 = ps_v2.tile([128, 256], F32, tag="pv2")
                    for ti, (xa, wp) in enumerate(terms_qk):
                        for j in range(3):
                            nc.tensor.matmul(
                                pv1[:],
                                xa[:, 2 * j : 2 * j + 2, 128 * st : 128 * (st + 1)],
                                wt["wv" + wp][:, 2 * j : 2 * j + 2, 0:512],
                                start=(ti == 0 and j == 0),
                                stop=(ti == 2 and j == 2),
                                perf_mode=DR,
                            )
                    for ti, (xa, wp) in enumerate(terms_qk):
                        for j in range(3):
                            nc.tensor.matmul(
                                pv2[:],
                                xa[:, 2 * j : 2 * j + 2, 128 * st : 128 * (st + 1)],
                                wt["wv" + wp][:, 2 * j : 2 * j + 2, 512:768],
                                start=(ti == 0 and j == 0),
                                stop=(ti == 2 and j == 2),
                                perf_mode=DR,
                            )
                    nc.scalar.activation(
                        vt[:, seq_tile, 0:512],
                        pv1[:],
                        mybir.ActivationFunctionType.Copy,
                        scale=EV_V,
                    )
                    nc.scalar.activation(
                        vt[:, seq_tile, 512:768],
                        pv2[:],
                        mybir.ActivationFunctionType.Copy,
                        scale=EV_V,
                    )

        # ---------------- Phase 2: attention (interleaved rounds) ----------------
        with ExitStack() as p2:
            ps_s = p2.enter_context(tc.tile_pool(name="ps_s", bufs=3, space="PSUM"))
            ps_c1 = p2.enter_context(tc.tile_pool(name="ps_c1", bufs=2, space="PSUM"))
            ps_c2 = p2.enter_context(tc.tile_pool(name="ps_c2", bufs=2, space="PSUM"))
            scd_p = p2.enter_context(tc.tile_pool(name="scd", bufs=2))
            ctx_p = p2.enter_context(tc.tile_pool(name="ctxs", bufs=2))
            small_p = p2.enter_context(tc.tile_pool(name="small", bufs=2))

            def round_tiles(u):
                tA = 4 * (u // 2) + (u % 2)  # diagonal key-tile
                return tA, tA + 2  # tB: masked (h=0) / allowed (h=1)

            def scores_for(t, win):
                qs = 128 * _mt(t)
                for ps, pw in _pieces(qs):
                    pss = ps_s.tile([128, 512], F32, tag="pss")
                    for j in range(3):
                        nc.tensor.matmul(
                            pss[:, 0:pw],
                            kt8[:, 2 * j : 2 * j + 2, 128 * t : 128 * (t + 1)],
                            qt8[:, 2 * j : 2 * j + 2, ps : ps + pw],
                            start=(j == 0),
                            stop=(j == 2),
                            perf_mode=DR,
                        )
                    dst = attnT[:, _OFF[t] + ps - qs : _OFF[t] + ps - qs + pw]
                    if ps == qs:  # strip piece: mask then exp
                        scd = scd_p.tile([128, 128], F32, tag="scd")
                        nc.vector.tensor_add(
                            scd[:], pss[:, 0:128], strip[:, 128 * win : 128 * (win + 1)]
                        )
                        nc.scalar.activation(dst, scd[:], EXP, scale=SC_EXP)
                    else:
                        nc.scalar.activation(dst, pss[:, 0:pw], EXP, scale=SC_EXP)

            for u in range(8):
                tA, tB = round_tiles(u)
                scores_for(tA, 0)
                scores_for(tB, 1)

                # context for q-tile u over key-tiles of rounds 0..u
                tiles = []
                for r in range(u + 1):
                    a, b = round_tiles(r)
                    tiles += [a, b]
                pc1 = ps_c1.tile([128, 512], F32, tag="pc1")
                pc2 = ps_c2.tile([128, 258], F32, tag="pc2")
                for idx, t in enumerate(tiles):
                    col = _OFF[t] + 128 * (u - _mt(t))
                    nc.tensor.matmul(
                        pc1[:],
                        attnT[:, col : col + 128],
                        vt[:, t, 0:512],
                        start=(idx == 0),
                        stop=(idx == len(tiles) - 1),
                    )
                for idx, t in enumerate(tiles):
                    col = _OFF[t] + 128 * (u - _mt(t))
                    nc.tensor.matmul(
                        pc2[:],
                        attnT[:, col : col + 128],
                        vt[:, t, 512:770],
                        start=(idx == 0),
                        stop=(idx == len(tiles) - 1),
                    )
                rinv = small_p.tile([128, 1], F32, tag="rinv")
                nc.vector.reciprocal(rinv[:], pc2[:, 256:257])
                ctx_sb = ctx_p.tile([128, D], F32, tag="ctxs")
                nc.vector.tensor_mul(
                    ctx_sb[:, 0:512], pc1[:], rinv[:].to_broadcast((128, 512))
                )
                nc.vector.tensor_mul(
                    ctx_sb[:, 512:768], pc2[:, 0:256], rinv[:].to_broadcast((128, 256))
                )
                nc.sync.dma_start(out_d[128 * u : 128 * (u + 1), :], ctx_sb[:])

    nc.compile()
    return nc


def _fp8_split(a, s):
    """Same-scale hi/lo fp8 split: a*s ~ hi + lo, both fp8 at scale s."""
    hi = (a * s).astype(float8_e4m3)
    lo = (a * s - hi.astype(np.float32)).astype(float8_e4m3)
    return hi, lo


def kernel(x, Wq, Wk, Wv):
    if "nc" not in _CACHE:
        _CACHE["nc"] = _build()
    nc = _CACHE["nc"]

    x = np.asarray(x, dtype=np.float32)
    # S^T layout: rows = key j (partitions), cols = query i; mask j > i
    diag = np.where(
        np.arange(128)[:, None] > np.arange(128)[None, :], NEG, 0.0
    ).astype(np.float32)

    whl = {}
    for name, W in (("wq", Wq), ("wk", Wk), ("wv", Wv)):
        wt = np.ascontiguousarray(np.asarray(W, dtype=np.float32).T)
        h, l = _fp8_split(wt, SW)
        whl[name + "h"] = np.ascontiguousarray(h)
        whl[name + "l"] = np.ascontiguousarray(l)

    # per-batch fp8 split of x^T in global order; per-core column permutation
    xsplit = []
    for b in range(BATCH):
        xsplit.append(_fp8_split(np.ascontiguousarray(x[b].T), SX))

    in_maps = []
    for c in range(8):
        b, h = c // 2, c % 2
        xh_g, xl_g = xsplit[b]
        order = []
        for sc in range(4):
            order += [4 * sc + h, 4 * sc + 2 + h, 4 * sc + 1 - h, 4 * sc + 3 - h]
        cols = np.concatenate([np.arange(128 * g, 128 * (g + 1)) for g in order])
        strip = np.concatenate(
            [diag, np.full((128, 128), NEG if h == 0 else 0.0, np.float32)], axis=1
        )
        in_maps.append(
            {
                "xh": np.ascontiguousarray(xh_g[:, cols]),
                "xl": np.ascontiguousarray(xl_g[:, cols]),
                **whl,
                "strip": np.ascontiguousarray(strip),
            }
        )

    res = run_bass_kernel_spmd(
        nc,
        in_maps,
        list(range(8)),
        trace=bool(int(os.environ.get("KERNEL_TRACE", "0"))),
    )
    _CACHE["last_results"] = res

    out = np.empty((BATCH, SEQ, D), np.float32)
    for c in range(8):
        b, h = c // 2, c % 2
        o = res.results[c]["out"]
        for lt in range(8):
            out[b, (2 * lt + h) * 128 : (2 * lt + h + 1) * 128] = o[
                128 * lt : 128 * (lt + 1)
            ]
    return out


# revision 11
# speedup vs baseline: 1.8759x; 1.1348x over previous
"""Causal single-head attention on 8 TRN2 NeuronCores — fp8 DoubleRow version.

Problem: x [4, 2048, 768] f32; Wq/Wk/Wv [768, 768] f32 (torch Linear layout).
  q/k/v = x @ W.T ; scores = q k^T causal-masked; attn = softmax(scores/sqrt(768));
  out = attn @ v.

Sharding: core c -> batch b = c//2, half h = c%2. The two cores of a batch
split the 16 query tiles (128 rows each) interleaved: core h owns global
q-tiles {2*lt + h}. The host permutes x^T's columns per-core so that within
each 512-column chunk the core's OWN two q-tiles come first:
  chunk sc columns = global tiles [4sc+h, 4sc+2+h, 4sc+1-h, 4sc+3-h].
This makes the Q projection a fixed [0:256] slice of each chunk (SPMD-uniform
across cores) while K/V simply inherit the permuted key order, which both
attention phases use consistently. Causal masking becomes per-core strip DATA:
by construction key-tile position parity determines diagonal / fully-masked /
fully-allowed, identical program on every core.

Numerics: all matmuls run in fp8-e4m3 with the DoubleRow perf mode (two
128-deep contraction tiles per instruction at 2x rate). Projections use a
same-scale hi+lo fp8 split of both x and W (x ~ (xh+xl)/16, W ~ (wh+wl)/512)
and accumulate the three significant cross terms in one PSUM group, giving
~1e-3 relative error. Scores quantize q,k to fp8 at scale 32 (error ~7e-3).
Softmax skips max-subtraction (scaled scores are O(+-2)) and folds all scale
constants into the exp scale. The context matmul runs in f32r from the
transposed attention weights written directly by exp (scores are computed
pre-transposed: S^T = K Q^T, keys on partitions), so no PE transposes exist.
The softmax row-sum comes from a ones-column appended to V, accumulated in
the same PSUM as the context, and is divided out at evacuation.

Attention is exact-causal at 128-key granularity: key-tile t is scored only
against the query range that can attend to it (plus one fully-masked 128-wide
block on even cores to keep the instruction stream uniform).
"""

import os
import sys
from contextlib import ExitStack

import numpy as np
from ml_dtypes import float8_e4m3

for _p in ("/opt/trn_rl_repo", "/root/.axon_site/_ro/trn_rl_repo"):
    if os.path.isdir(_p) and _p not in sys.path:
        sys.path.append(_p)

import concourse.mybir as mybir  # noqa: E402
import concourse.tile as tile  # noqa: E402
from concourse import bacc  # noqa: E402
from concourse.bass_utils import run_bass_kernel_spmd  # noqa: E402

F32 = mybir.dt.float32
F32R = mybir.dt.float32r
F8 = mybir.dt.float8e4
DR = mybir.MatmulPerfMode.DoubleRow
EXP = mybir.ActivationFunctionType.Exp

BATCH = 4
SEQ = 2048
D = 768
NQ = 1024  # query rows per core
NEG = -1e30

SX = 16.0  # x fp8 scale
SW = 512.0  # W fp8 scale
SQK = 32.0  # q/k fp8 scale
S_PROJ = SX * SW  # PSUM scale of projections
EV_QK = SQK / S_PROJ  # evac scale PSUM -> q/k fp8
EV_V = 1.0 / S_PROJ  # evac scale PSUM -> v f32
SC_EXP = 1.0 / (float(np.sqrt(np.float32(D))) * SQK * SQK)

# key-tile position p within a chunk -> min local q-tile offset (2sc + MOFF[p])
MOFF = (0, 1, 0, 1)


def _mt(t):  # min local q-tile index attending to key-tile t
    return 2 * (t // 4) + MOFF[t % 4]


_W = [NQ - 128 * _mt(t) for t in range(16)]  # scored q-width per key-tile
_OFF = [0] * 16  # attnT column offset per key-tile
for _t in range(1, 16):
    _OFF[_t] = _OFF[_t - 1] + _W[_t - 1]
ATTNT_COLS = _OFF[15] + _W[15]  # 9216

_CACHE = {}


def _pieces(qs):
    """Split q-range [qs, NQ) into the strip piece (128) + <=512 chunks."""
    out = [(qs, 128)]
    pos = qs + 128
    while pos < NQ:
        w = min(512, NQ - pos)
        out.append((pos, w))
        pos += w
    return out


def _build():
    nc = bacc.Bacc("TRN2", target_bir_lowering=False, debug=False, num_devices=8)
    xh_d = nc.declare_dram_parameter("xh", [D, SEQ], F8, isOutput=False)
    xl_d = nc.declare_dram_parameter("xl", [D, SEQ], F8, isOutput=False)
    w_d = {}
    for name in ("wqh", "wkh", "wvh", "wvl"):
        w_d[name] = nc.declare_dram_parameter(name, [D, D], F8, isOutput=False)
    strip_d = nc.declare_dram_parameter("strip", [128, 256], F32, isOutput=False)
    out_d = nc.declare_dram_parameter("out", [NQ, D], F32, isOutput=True)

    # Rotate input DMAs across engine DGE queues (issue-side seq cost).
    _dma_i = [0]

    def dma_in(dst, src):
        eng = (nc.sync, nc.scalar)[_dma_i[0] % 2]
        eng.dma_start(dst, src)
        _dma_i[0] += 1

    with tile.TileContext(nc) as tc, ExitStack() as ctx:
        persist = ctx.enter_context(tc.tile_pool(name="persist", bufs=1))

        strip = persist.tile([128, 256], F32)
        kt8 = persist.tile([128, 6, SEQ], F8)  # K^T fp8 (scale SQK)
        qt8 = persist.tile([128, 6, NQ], F8)  # Q^T fp8 (scale SQK)
        vt = persist.tile([128, 16, 776], F32R)  # V (+ones col 768)
        attnT = persist.tile([128, ATTNT_COLS], F32R)  # exp(S^T) blocks

        # Q/K use a 2-term split (x hi+lo vs W hi only): their error feeds the
        # scores, which already carry the q/k fp8 quantization noise. V keeps
        # all 3 terms since its error lands directly in the output.
        wt = {}
        for name in ("wqh", "wkh", "wvh", "wvl"):
            wt[name] = persist.tile([128, 6, D], F8, name=name)

        ones = persist.tile([128, 1], F32)
        nc.vector.memset(ones[:], 1.0)
        nc.vector.tensor_copy(vt[:, :, 768:770], ones[:].to_broadcast((128, 16, 2)))

        def dma_w(name):
            dma_in(
                wt[name][:],
                w_d[name][:].rearrange("(ko p) o -> p ko o", p=128),
            )

        # ---------------- Phase 1: projections ----------------
        with ExitStack() as p1:
            xc_p = p1.enter_context(tc.tile_pool(name="xc", bufs=2))
            ps_q = p1.enter_context(tc.tile_pool(name="ps_q", bufs=2, space="PSUM"))
            ps_k = p1.enter_context(tc.tile_pool(name="ps_k", bufs=2, space="PSUM"))
            ps_v1 = p1.enter_context(tc.tile_pool(name="ps_v1", bufs=2, space="PSUM"))
            ps_v2 = p1.enter_context(tc.tile_pool(name="ps_v2", bufs=2, space="PSUM"))

            dma_w("wqh")

            for sc in range(4):
                xh = xc_p.tile([128, 6, 512], F8, tag="xh")
                xl = xc_p.tile([128, 6, 512], F8, tag="xl")
                # pair-granular loads so the first Q group starts early
                for j in range(3):
                    dma_in(
                        xh[:, 2 * j : 2 * j + 2, :],
                        xh_d[
                            256 * j : 256 * (j + 1), 512 * sc : 512 * (sc + 1)
                        ].rearrange("(ko p) s -> p ko s", p=128),
                    )
                for j in range(3):
                    dma_in(
                        xl[:, 2 * j : 2 * j + 2, :],
                        xl_d[
                            256 * j : 256 * (j + 1), 512 * sc : 512 * (sc + 1)
                        ].rearrange("(ko p) s -> p ko s", p=128),
                    )
                if sc == 0:
                    dma_w("wkh")
                    dma_w("wvh")
                    dma_w("wvl")
                    dma_in(strip[:], strip_d[:])

                terms_qk = ((xh, "h"), (xl, "h"))
                terms_v = ((xh, "h"), (xh, "l"), (xl, "h"))

                # Q: own q-tiles live in chunk cols [0:256]
                for oo in range(6):
                    pq = ps_q.tile([128, 256], F32, tag="pq")
                    for ti, (xa, wp) in enumerate(terms_qk):
                        for j in range(3):
                            nc.tensor.matmul(
                                pq[:],
                                wt["wq" + wp][:, 2 * j : 2 * j + 2, 128 * oo : 128 * (oo + 1)],
                                xa[:, 2 * j : 2 * j + 2, 0:256],
                                start=(ti == 0 and j == 0),
                                stop=(ti == len(terms_qk) - 1 and j == 2),
                                perf_mode=DR,
                            )
                    nc.vector.tensor_scalar_mul(
                        qt8[:, oo, 256 * sc : 256 * (sc + 1)], pq[:], EV_QK
                    )

                # K^T
                for oo in range(6):
                    pk = ps_k.tile([128, 512], F32, tag="pk")
                    for ti, (xa, wp) in enumerate(terms_qk):
                        for j in range(3):
                            nc.tensor.matmul(
                                pk[:],
                                wt["wk" + wp][:, 2 * j : 2 * j + 2, 128 * oo : 128 * (oo + 1)],
                                xa[:, 2 * j : 2 * j + 2, :],
                                start=(ti == 0 and j == 0),
                                stop=(ti == len(terms_qk) - 1 and j == 2),
                                perf_mode=DR,
                            )
                    nc.scalar.activation(
                        kt8[:, oo, 512 * sc : 512 * (sc + 1)],
                        pk[:],
                        mybir.ActivationFunctionType.Copy,
                        scale=EV_QK,
                    )

                # V rows (natural layout), d_out in 512+256
                for st in range(4):
                    seq_tile = 4 * sc + st
                    pv1 = ps_v1.tile([128, 512], F32, tag="pv1")
                    pv2 = ps_v2.tile([128, 256], F32, tag="pv2")
                    for ti, (xa, wp) in enumerate(terms_v):
                        for j in range(3):
                            nc.tensor.matmul(
                                pv1[:],
                                xa[:, 2 * j : 2 * j + 2, 128 * st : 128 * (st + 1)],
                                wt["wv" + wp][:, 2 * j : 2 * j + 2, 0:512],
                                start=(ti == 0 and j == 0),
                                stop=(ti == 2 and j == 2),
                                perf_mode=DR,
                            )
                    for ti, (xa, wp) in enumerate(terms_v):
                        for j in range(3):
                            nc.tensor.matmul(
                                pv2[:],
                                xa[:, 2 * j : 2 * j + 2, 128 * st : 128 * (st + 1)],
                                wt["wv" + wp][:, 2 * j : 2 * j + 2, 512:768],
                                start=(ti == 0 and j == 0),
                                stop=(ti == 2 and j == 2),
                                perf_mode=DR,
                            )
                    nc.scalar.activation(
                        vt[:, seq_tile, 0:512],
                        pv1[:],
                        mybir.ActivationFunctionType.Copy,
                        scale=EV_V,
                    )
                    nc.scalar.activation(
                        vt[:, seq_tile, 512:768],
                        pv2[:],
                        mybir.ActivationFunctionType.Copy,
                        scale=EV_V,
                    )

        # ---------------- Phase 2: attention (interleaved rounds) ----------------
        with ExitStack() as p2:
            ps_s = p2.enter_context(tc.tile_pool(name="ps_s", bufs=3, space="PSUM"))
            ps_c1 = p2.enter_context(tc.tile_pool(name="ps_c1", bufs=2, space="PSUM"))
            ps_c2 = p2.enter_context(tc.tile_pool(name="ps_c2", bufs=2, space="PSUM"))
            scd_p = p2.enter_context(tc.tile_pool(name="scd", bufs=2))
            ctx_p = p2.enter_context(tc.tile_pool(name="ctxs", bufs=2))
            small_p = p2.enter_context(tc.tile_pool(name="small", bufs=2))

            def round_tiles(u):
                tA = 4 * (u // 2) + (u % 2)  # diagonal key-tile
                return tA, tA + 2  # tB: masked (h=0) / allowed (h=1)

            def scores_for(t, win):
                qs = 128 * _mt(t)
                for ps, pw in _pieces(qs):
                    pss = ps_s.tile([128, 512], F32, tag="pss")
                    for j in range(3):
                        nc.tensor.matmul(
                            pss[:, 0:pw],
                            kt8[:, 2 * j : 2 * j + 2, 128 * t : 128 * (t + 1)],
                            qt8[:, 2 * j : 2 * j + 2, ps : ps + pw],
                            start=(j == 0),
                            stop=(j == 2),
                            perf_mode=DR,
                        )
                    dst = attnT[:, _OFF[t] + ps - qs : _OFF[t] + ps - qs + pw]
                    if ps == qs:  # strip piece: mask then exp
                        scd = scd_p.tile([128, 128], F32, tag="scd")
                        nc.vector.tensor_add(
                            scd[:], pss[:, 0:128], strip[:, 128 * win : 128 * (win + 1)]
                        )
                        nc.scalar.activation(dst, scd[:], EXP, scale=SC_EXP)
                    else:
                        nc.scalar.activation(dst, pss[:, 0:pw], EXP, scale=SC_EXP)

            for u in range(8):
                tA, tB = round_tiles(u)
                scores_for(tA, 0)
                scores_for(tB, 1)

                # context for q-tile u over key-tiles of rounds 0..u
                tiles = []
                for r in range(u + 1):
                    a, b = round_tiles(r)
                    tiles += [a, b]
                pc1 = ps_c1.tile([128, 512], F32, tag="pc1")
                pc2 = ps_c2.tile([128, 258], F32, tag="pc2")
                # pc2 first: its rowsum column feeds the reciprocal, which then
                # overlaps the pc1 accumulation
                for idx, t in enumerate(tiles):
                    col = _OFF[t] + 128 * (u - _mt(t))
                    nc.tensor.matmul(
                        pc2[:],
                        attnT[:, col : col + 128],
                        vt[:, t, 512:770],
                        start=(idx == 0),
                        stop=(idx == len(tiles) - 1),
                    )
                rinv = small_p.tile([128, 1], F32, tag="rinv")
                nc.vector.reciprocal(rinv[:], pc2[:, 256:257])
                ctx_sb = ctx_p.tile([128, D], F32, tag="ctxs")
                nc.vector.tensor_mul(
                    ctx_sb[:, 512:768], pc2[:, 0:256], rinv[:].to_broadcast((128, 256))
                )
                for idx, t in enumerate(tiles):
                    col = _OFF[t] + 128 * (u - _mt(t))
                    nc.tensor.matmul(
                        pc1[:],
                        attnT[:, col : col + 128],
                        vt[:, t, 0:512],
                        start=(idx == 0),
                        stop=(idx == len(tiles) - 1),
                    )
                nc.vector.tensor_mul(
                    ctx_sb[:, 0:512], pc1[:], rinv[:].to_broadcast((128, 512))
                )
                nc.sync.dma_start(
                    out_d[128 * u : 128 * (u + 1), 512:768], ctx_sb[:, 512:768]
                )
                nc.sync.dma_start(
                    out_d[128 * u : 128 * (u + 1), 0:512], ctx_sb[:, 0:512]
                )

    nc.compile()
    return nc


def _fp8_split(a, s):
    """Same-scale hi/lo fp8 split: a*s ~ hi + lo, both fp8 at scale s."""
    hi = (a * s).astype(float8_e4m3)
    lo = (a * s - hi.astype(np.float32)).astype(float8_e4m3)
    return hi, lo


def kernel(x, Wq, Wk, Wv):
    if "nc" not in _CACHE:
        _CACHE["nc"] = _build()
    nc = _CACHE["nc"]

    x = np.asarray(x, dtype=np.float32)
    # S^T layout: rows = key j (partitions), cols = query i; mask j > i
    diag = np.where(
        np.arange(128)[:, None] > np.arange(128)[None, :], NEG, 0.0
    ).astype(np.float32)

    whl = {}
    for name, W in (("wq", Wq), ("wk", Wk), ("wv", Wv)):
        wt = np.ascontiguousarray(np.asarray(W, dtype=np.float32).T)
        h, l = _fp8_split(wt, SW)
        whl[name + "h"] = np.ascontiguousarray(h)
        if name == "wv":
            whl[name + "l"] = np.ascontiguousarray(l)

    # per-batch fp8 split of x^T in global order; per-core column permutation
    xsplit = []
    for b in range(BATCH):
        xsplit.append(_fp8_split(np.ascontiguousarray(x[b].T), SX))

    in_maps = []
    for c in range(8):
        b, h = c // 2, c % 2
        xh_g, xl_g = xsplit[b]
        order = []
        for sc in range(4):
            order += [4 * sc + h, 4 * sc + 2 + h, 4 * sc + 1 - h, 4 * sc + 3 - h]
        cols = np.concatenate([np.arange(128 * g, 128 * (g + 1)) for g in order])
        strip = np.concatenate(
            [diag, np.full((128, 128), NEG if h == 0 else 0.0, np.float32)], axis=1
        )
        in_maps.append(
            {
                "xh": np.ascontiguousarray(xh_g[:, cols]),
                "xl": np.ascontiguousarray(xl_g[:, cols]),
                **whl,
                "strip": np.ascontiguousarray(strip),
            }
        )

    res = run_bass_kernel_spmd(
        nc,
        in_maps,
        list(range(8)),
        trace=bool(int(os.environ.get("KERNEL_TRACE", "0"))),
    )
    _CACHE["last_results"] = res

    out = np.empty((BATCH, SEQ, D), np.float32)
    for c in range(8):
        b, h = c // 2, c % 2
        o = res.results[c]["out"]
        for lt in range(8):
            out[b, (2 * lt + h) * 128 : (2 * lt + h + 1) * 128] = o[
                128 * lt : 128 * (lt + 1)
            ]
    return out
